# revision 30
# baseline (speedup 1.0000x reference)
"""Cross-attention kernel for 8 Trainium2 NeuronCores.

Sharding: 8 cores = 4 batches x 2 head-groups (6 heads each).
Per core (b, hg), with all activations pre-transposed on host:
  qT = (Wq_hg*scale).T' @ xqT   [384, 2048]   (weights column-split)
  kT = Wk_hg' @ xkT             [384, 2048]
  v  = xvT' @ Wv_hg.T           [2048, 384]  (+ ones column per head)
  per head h: lt = k_h qT_h     [2048k, 2048q] (logits transposed)
              p  = exp(lt)      (no max-subtraction: logits are O(1))
              [x; d] = [v_h|1].T @ p   -> x rows 0..63, denominators row 64
              xn = x * (1/d)    (partition-broadcast of 1/d)
  outT_partial = Wo_hg.T' @ xn  [768, 2048]
Host: out[b] = (partial[2b] + partial[2b+1]).T + bo.

All matmuls bf16 inputs with fp32 PSUM accumulation.

v2 (same math as baseline, denser pipeline):
- reciprocal_approx_fast (single custom-DVE op) replaces InstReciprocal
  (3.3us x24 = 80us in the baseline trace).
- gpsimd partition_broadcast replaces the DRAM-bounce broadcast DMA.
- Fine-grained emission: per (pair, chunk) unit, 8 "ktp blocks" of
  [QKT 4mm; 2 exps; fillers; AV of previous ktp 4mm]. The v/pair-1/2
  projections and the out-projections ride as fillers inside early
  units so the PE stream is dense and ACT (exp, ~214us total - the
  steady-state bottleneck) is never starved.
- Input DMAs ordered wq,wk,xq,xk first so projections start early.
"""

import sys

import numpy as np

for _p in ("/opt/trn_rl_repo",):
    if _p not in sys.path:
        sys.path.insert(0, _p)

B, NQ, NK, C = 4, 2048, 2048, 768
H, DH = 12, 64
HPC, HB = 6, 384  # heads per core, head-block width
P = 128
KT = C // P  # 6 contraction k-tiles for projections
QCH = 512  # query-chunk width
NCH = NQ // QCH  # 4 query chunks
NKT = NK // P  # 16 key tiles
SCALE = DH**-0.5  # folded into Wq on host (exactly 0.125)
VW = DH + 1  # v block width per head incl. ones column

_prog = None
DEBUG = False


def _build():
    from contextlib import ExitStack

    import concourse.bass as bass
    import concourse.tile as tile
    from concourse import library_config, mybir
    from concourse.bacc import Bacc

    f32 = mybir.dt.float32
    bf16 = mybir.dt.bfloat16
    EXP = mybir.ActivationFunctionType.Exp

    nc = Bacc()
    xq_d = nc.declare_dram_parameter("xq", [C, NQ], bf16, isOutput=False)
    xk_d = nc.declare_dram_parameter("xk", [C, NK], bf16, isOutput=False)
    xv_d = nc.declare_dram_parameter("xv", [C, NK], bf16, isOutput=False)
    wq_d = nc.declare_dram_parameter("wq", [C, HB], bf16, isOutput=False)
    wk_d = nc.declare_dram_parameter("wk", [C, HB], bf16, isOutput=False)
    wv_d = nc.declare_dram_parameter("wv", [C, HB], bf16, isOutput=False)
    wo_d = nc.declare_dram_parameter("wo", [HB, C], bf16, isOutput=False)
    out_d = nc.declare_dram_parameter("out", [C, NQ], f32, isOutput=True)
    if DEBUG:
        dbg_d = {
            "dbg_d": nc.declare_dram_parameter("dbg_d", [1, QCH], f32, isOutput=True),
            "dbg_r": nc.declare_dram_parameter("dbg_r", [1, QCH], f32, isOutput=True),
            "dbg_rb": nc.declare_dram_parameter("dbg_rb", [DH, QCH], f32, isOutput=True),
            "dbg_qT": nc.declare_dram_parameter("dbg_qT", [P, NQ], f32, isOutput=True),
            "dbg_xn": nc.declare_dram_parameter("dbg_xn", [P, NQ], f32, isOutput=True),
        }

    with tile.TileContext(nc) as tc, ExitStack() as ctx:
        const = ctx.enter_context(tc.tile_pool(name="const", bufs=1))
        xin = ctx.enter_context(tc.tile_pool(name="xin", bufs=1))
        qk = ctx.enter_context(tc.tile_pool(name="qk", bufs=1))
        pp = ctx.enter_context(tc.tile_pool(name="pp", bufs=18))
        xnp = ctx.enter_context(tc.tile_pool(name="xnp", bufs=1))
        small = ctx.enter_context(tc.tile_pool(name="small", bufs=2))
        ost = ctx.enter_context(tc.tile_pool(name="ost", bufs=3))
        proj_ps = ctx.enter_context(tc.tile_pool(name="proj_ps", bufs=2, space="PSUM"))
        lt_ps = ctx.enter_context(tc.tile_pool(name="lt_ps", bufs=2, space="PSUM"))
        x_ps = ctx.enter_context(tc.tile_pool(name="x_ps", bufs=2, space="PSUM"))

        # gpsimd ucode library for partition_broadcast
        nc.gpsimd.load_library(library_config.attn)

        # ---- input DMAs, priority order (the sync HWDGE ring drains
        # roughly in order): pair-0 q/k weight slices, xq, xk, remaining
        # q/k weights, wv, xv, wo (needed last).
        wq_s = const.tile([P, KT, HB], bf16, tag="wq")
        wk_s = const.tile([P, KT, HB], bf16, tag="wk")
        wq_r = wq_d.rearrange("(k p) m -> p k m", p=P)
        wk_r = wk_d.rearrange("(k p) m -> p k m", p=P)
        nc.sync.dma_start(out=wq_s[:, :, 0:P], in_=wq_r[:, :, 0:P])
        nc.sync.dma_start(out=wk_s[:, :, 0:P], in_=wk_r[:, :, 0:P])

        xq_t, xk_t, xv_t = [], [], []
        for name, dram, lst in (("xq", xq_d, xq_t), ("xk", xk_d, xk_t)):
            for k in range(KT):
                t = xin.tile([P, NQ], bf16, tag=f"{name}_{k}", name=f"{name}_{k}")
                nc.sync.dma_start(
                    out=t, in_=dram.rearrange("(k p) m -> p k m", p=P)[:, k, :]
                )
                lst.append(t)

        nc.sync.dma_start(out=wq_s[:, :, P:HB], in_=wq_r[:, :, P:HB])
        nc.sync.dma_start(out=wk_s[:, :, P:HB], in_=wk_r[:, :, P:HB])
        wv_s = const.tile([P, KT, HB], bf16, tag="wv")
        wo_s = const.tile([P, HB // P, C], bf16, tag="wo")
        nc.sync.dma_start(out=wv_s, in_=wv_d.rearrange("(k p) m -> p k m", p=P))
        for k in range(KT):
            t = xin.tile([P, NQ], bf16, tag=f"xv_{k}", name=f"xv_{k}")
            nc.sync.dma_start(
                out=t, in_=xv_d.rearrange("(k p) m -> p k m", p=P)[:, k, :]
            )
            xv_t.append(t)
        nc.sync.dma_start(out=wo_s, in_=wo_d.rearrange("(k p) m -> p k m", p=P))

        # v with a ones column per head: [128, kt, head, 65]
        v_s = const.tile([P, NKT, HPC, VW], bf16, tag="v")
        nc.vector.memset(v_s[:, :, :, DH : DH + 1], 1.0)

        qT_t = [qk.tile([P, NQ], bf16, tag=f"qT{i}", name=f"qT{i}") for i in range(3)]
        kT_t = [qk.tile([P, NQ], bf16, tag=f"kT{i}", name=f"kT{i}") for i in range(3)]
        xn_t = [xnp.tile([P, NQ], bf16, tag=f"xn{i}", name=f"xn{i}") for i in range(3)]

        # ---- small emitters used as PE fillers
        def projqk_block(w_s, src, dst, mt, j4):
            def f():
                ps = proj_ps.tile([P, QCH], f32, tag="proj", name="ps")
                for k in range(KT):
                    nc.tensor.matmul(
                        ps,
                        w_s[:, k, mt * P : (mt + 1) * P],
                        src[k][:, j4 * QCH : (j4 + 1) * QCH],
                        start=(k == 0),
                        stop=(k == KT - 1),
                    )
                nc.vector.tensor_copy(dst[:, j4 * QCH : (j4 + 1) * QCH], ps)

            return f

        def vproj_block(kt):
            def f():
                ps = proj_ps.tile([P, HB], f32, tag="proj", name="ps")
                for k in range(KT):
                    nc.tensor.matmul(
                        ps,
                        xv_t[k][:, kt * P : (kt + 1) * P],
                        wv_s[:, k, :],
                        start=(k == 0),
                        stop=(k == KT - 1),
                    )
                nc.vector.tensor_copy(
                    v_s[:, kt, :, 0:DH], ps.rearrange("p (h m) -> p h m", m=DH)
                )

            return f

        def outproj_block(j4, mt):
            def f():
                ps = proj_ps.tile([P, QCH], f32, tag="proj", name="ps")
                for k3 in range(HB // P):
                    nc.tensor.matmul(
                        ps,
                        wo_s[:, k3, mt * P : (mt + 1) * P],
                        xn_t[k3][:, j4 * QCH : (j4 + 1) * QCH],
                        start=(k3 == 0),
                        stop=(k3 == HB // P - 1),
                    )
                o = ost.tile([P, QCH], f32, tag="o", name="o")
                nc.vector.tensor_copy(o, ps)
                nc.sync.dma_start(
                    out=out_d[mt * P : (mt + 1) * P, j4 * QCH : (j4 + 1) * QCH], in_=o
                )

            return f

        # ---- attention unit = one (pair, chunk): 8 ktp blocks of
        # [QKT 4mm; 2 exps; AV block of the PREVIOUS unit; fillers].
        # The whole AV of unit u rides inside unit u+1 (uniform unit lag),
        # so AVs never gate on in-flight exps and xd PSUM slots alternate
        # between non-adjacent units.
        rows = (slice(0, DH), slice(DH, 2 * DH))
        av_carry = [[]]  # 8 pending AV-block emitters from the previous unit

        def av_block(p3, j4, ktp, p_pair, xd_pair):
            qsl = slice(j4 * QCH, (j4 + 1) * QCH)

            def f():
                for hh in range(2):
                    h = 2 * p3 + hh
                    for u in range(2):
                        kt = 2 * ktp + u
                        nc.tensor.matmul(
                            xd_pair[hh],
                            v_s[:, kt, h, :],
                            p_pair[hh][:, u * QCH : (u + 1) * QCH],
                            start=(kt == 0),
                            stop=(kt == NKT - 1),
                        )
                if ktp == NKT // 2 - 1:
                    # normalize both heads, chains interleaved so the two
                    # heads pipeline across DVE and GpSimd.
                    # custom-DVE ops mishandle nonzero input base partition:
                    # copy the denominator row to partition 0 first.
                    # h1 first: its path has the extra cross-partition DMA hop
                    dcps, rs, rbs = {}, {}, {}
                    for hh in (1, 0):
                        dcps[hh] = small.tile([1, QCH], f32, tag="dcp", name="dcp")
                        nc.vector.tensor_copy(dcps[hh], xd_pair[hh][DH : DH + 1, :])
                    for hh in (1, 0):
                        rs[hh] = small.tile([1, QCH], f32, tag="r", name="r")
                        nc.vector.reciprocal_approx_fast(rs[hh], dcps[hh])
                    for hh in (1, 0):
                        rbs[hh] = small.tile([DH, QCH], f32, tag="rb", name="rb")
                        nc.gpsimd.partition_broadcast(rbs[hh], rs[hh])
                    tmp = small.tile([DH, QCH], bf16, tag="tmp", name="tmp")
                    nc.vector.tensor_mul(tmp, xd_pair[1][0:DH, :], rbs[1])
                    nc.vector.tensor_mul(
                        xn_t[p3][0:DH, qsl], xd_pair[0][0:DH, :], rbs[0]
                    )
                    nc.sync.dma_start(out=xn_t[p3][DH : 2 * DH, qsl], in_=tmp)

            return f

        def attn_unit(p3, j4, fillers, chase=False):
            """fillers: 8 lists of emitters, one per ktp block, emitted
            after the previous unit's AV block. chase=True additionally
            runs this unit's own AVs at 1-ktp lag (for the final unit, so
            only one AV block remains after the last exp)."""
            qsl = slice(j4 * QCH, (j4 + 1) * QCH)
            xd_pair = [
                x_ps.tile([DH + 1, QCH], f32, tag="x", name=f"xd{hh}")
                for hh in range(2)
            ]
            prev_av = av_carry[0]
            my_av = []
            for ktp in range(NKT // 2):
                lts = [
                    lt_ps.tile([P, 2 * QCH], f32, tag="lt", name=f"lt{hh}")
                    for hh in range(2)
                ]
                for u in range(2):
                    kt = 2 * ktp + u
                    for hh in range(2):
                        nc.tensor.matmul(
                            lts[hh][:, u * QCH : (u + 1) * QCH],
                            kT_t[p3][rows[hh], kt * P : (kt + 1) * P],
                            qT_t[p3][rows[hh], qsl],
                            start=True,
                            stop=True,
                        )
                p_pair = []
                for hh in range(2):
                    pt = pp.tile([P, 2 * QCH], bf16, tag="p", name="pt")
                    nc.scalar.activation(pt, lts[hh], EXP)
                    p_pair.append(pt)
                # previous unit's AVs on blocks 0..6 (two on block 0) so its
                # normalize completes well before the next unit reuses xd
                if ktp == 0:
                    for g in prev_av[0:2]:
                        g()
                elif ktp <= 6 and ktp + 1 < len(prev_av):
                    prev_av[ktp + 1]()
                my_av.append(av_block(p3, j4, ktp, p_pair, xd_pair))
                if chase and ktp > 0:
                    my_av[ktp - 1]()
                for f in fillers[ktp]:
                    f()
            av_carry[0] = my_av[-1:] if chase else my_av

        def flush_av():
            for f in av_carry[0]:
                f()
            av_carry[0] = []

        def spread(emitters):
            """Distribute a list of emitters over 8 ktp blocks, in order."""
            out = [[] for _ in range(8)]
            for i, e in enumerate(emitters):
                out[min(i * 8 // max(len(emitters), 1), 7)].append(e)
            return out

        # ---- emission schedule. Region-level deps let QKT(p,c,ktp) start
        # once qT[p] chunk c and kT[p] key-block ktp//2 are projected, so
        # only qp0(j0)+kp0(j0) run before the first attention unit; every
        # other projection block rides as a filler.
        def qp(p3, j4):
            return projqk_block(wq_s, xq_t, qT_t[p3], p3, j4)

        def kp(p3, j4):
            return projqk_block(wk_s, xk_t, kT_t[p3], p3, j4)

        qp(0, 0)()
        kp(0, 0)()

        # unit (p0,c0): kp0 key-blocks just in time (QKT ktp needs
        # kp0(ktp//2)); v-projection at blocks 4-7 (xv arrives late; the
        # AVs consuming it ride in unit (p1,c0)); pair-1 j0 at the end.
        u0 = [[] for _ in range(8)]
        u0[1].append(kp(0, 1))
        u0[3].append(kp(0, 2))
        u0[5].append(kp(0, 3))
        for i in range(8):
            u0[4 + i // 2].append(vproj_block(i))
        u0[6].append(qp(1, 0))
        u0[7].append(kp(1, 0))
        attn_unit(0, 0, u0)

        # unit (p1,c0): rest of the v-projection just in time for the
        # AV(p0,c0) blocks riding here (AV ktp k at block max(k-1,0) needs
        # v tiles 2k,2k+1), rest of kp1, pair-2 j0 at the end
        u1 = [[] for _ in range(8)]
        for i in range(8, 16):
            u1[(i - 8) // 2].append(vproj_block(i))
        u1[1].append(kp(1, 1))
        u1[3].append(kp(1, 2))
        u1[5].append(kp(1, 3))
        u1[6].append(qp(2, 0))
        u1[7].append(kp(2, 0))
        attn_unit(1, 0, u1)

        # unit (p2,c0): rest of kp2, plus q projections for chunk 1
        u2 = [[] for _ in range(8)]
        u2[1].append(kp(2, 1))
        u2[3].append(kp(2, 2))
        u2[5].append(kp(2, 3))
        u2[6].append(qp(0, 1))
        u2[7].append(qp(1, 1))
        attn_unit(2, 0, u2)

        # chunks 1..3. normalize(p2,c) lands in unit (p0,c+1) block 6, so
        # out-proj(c) rides in unit (p1,c+1) blocks 1..6. q projections for
        # later chunks trickle in one unit ahead of their consumer.
        for j4 in range(1, NCH):
            ua = [[] for _ in range(8)]
            attn_unit(0, j4, ua)
            ub = [[] for _ in range(8)]
            ub[0].append(qp(2, j4))
            # in the last chunk keep out-proj mts 3..5 of chunk 2 for the
            # flush, so the PE stays warm through the final normalize
            n_op = 3 if j4 == NCH - 1 else C // P
            for mt in range(n_op):
                ub[1 + mt].append(outproj_block(j4 - 1, mt))
            attn_unit(1, j4, ub, chase=(j4 == NCH - 1))
            uc = [[] for _ in range(8)]
            if j4 < NCH - 1:
                uc[6].append(qp(0, j4 + 1))
                uc[7].append(qp(1, j4 + 1))
            attn_unit(2, j4, uc, chase=(j4 == NCH - 1))

        # flush: AV+normalize of the last unit overlapped with the held-back
        # chunk-2 out-projs (PE warm-keepers), then the final chunk's out-proj
        flush_av()
        for mt in range(3, C // P):
            outproj_block(NCH - 2, mt)()
        for mt in range(C // P):
            outproj_block(NCH - 1, mt)()

        if DEBUG:
            for nm, t in (("dbg_qT", qT_t[0]), ("dbg_xn", xn_t[0])):
                dc = ost.tile([P, NQ], f32, tag="dbgc", name="dbgc", bufs=1)
                nc.vector.tensor_copy(dc, t)
                nc.sync.dma_start(out=dbg_d[nm][:, :], in_=dc)

    nc.finalize()
    return nc


def _get_prog():
    global _prog
    if _prog is None:
        _prog = _build()
    return _prog


def _shard_inputs(query, key, value, Wq, Wk, Wv, Wo):
    from ml_dtypes import bfloat16

    in_maps = []
    for core in range(8):
        b, hg = core // 2, core % 2
        sl = slice(hg * HB, (hg + 1) * HB)
        in_maps.append(
            {
                "xq": np.ascontiguousarray(query[b].T).astype(bfloat16),
                "xk": np.ascontiguousarray(key[b].T).astype(bfloat16),
                "xv": np.ascontiguousarray(value[b].T).astype(bfloat16),
                "wq": np.ascontiguousarray((Wq[sl, :] * SCALE).T).astype(bfloat16),
                "wk": np.ascontiguousarray(Wk[sl, :].T).astype(bfloat16),
                "wv": np.ascontiguousarray(Wv[sl, :].T).astype(bfloat16),
                "wo": np.ascontiguousarray(Wo[:, sl].T).astype(bfloat16),
            }
        )
    return in_maps


def kernel(query, key, value, Wq, Wk, Wv, Wo, bo):
    query, key, value = np.asarray(query), np.asarray(key), np.asarray(value)
    Wq, Wk, Wv, Wo = np.asarray(Wq), np.asarray(Wk), np.asarray(Wv), np.asarray(Wo)
    bo = np.asarray(bo).astype(np.float32)

    from concourse.bass_utils import run_bass_kernel_spmd

    nc = _get_prog()
    in_maps = _shard_inputs(query, key, value, Wq, Wk, Wv, Wo)
    res = run_bass_kernel_spmd(nc, in_maps, list(range(8))).results

    out = np.empty((B, NQ, C), np.float32)
    for b in range(B):
        acc = res[2 * b]["out"].astype(np.float32) + res[2 * b + 1]["out"].astype(
            np.float32
        )
        out[b] = acc.T + bo[None, :]
    return out


# revision 31
# speedup vs baseline: 1.0040x; 1.0040x over previous
"""Cross-attention kernel for 8 Trainium2 NeuronCores.

Sharding: 8 cores = 4 batches x 2 head-groups (6 heads each).
Per core (b, hg), with all activations pre-transposed on host:
  qT = (Wq_hg*scale).T' @ xqT   [384, 2048]   (weights column-split)
  kT = Wk_hg' @ xkT             [384, 2048]
  v  = xvT' @ Wv_hg.T           [2048, 384]  (+ ones column per head)
  per head h: lt = k_h qT_h     [2048k, 2048q] (logits transposed)
              p  = exp(lt)      (no max-subtraction: logits are O(1))
              [x; d] = [v_h|1].T @ p   -> x rows 0..63, denominators row 64
              xn = x * (1/d)    (partition-broadcast of 1/d)
  outT_partial = Wo_hg.T' @ xn  [768, 2048]
Host: out[b] = (partial[2b] + partial[2b+1]).T + bo.

All matmuls bf16 inputs with fp32 PSUM accumulation.

v2 (same math as baseline, denser pipeline):
- reciprocal_approx_fast (single custom-DVE op) replaces InstReciprocal
  (3.3us x24 = 80us in the baseline trace).
- gpsimd partition_broadcast replaces the DRAM-bounce broadcast DMA.
- Fine-grained emission: per (pair, chunk) unit, 8 "ktp blocks" of
  [QKT 4mm; 2 exps; fillers; AV of previous ktp 4mm]. The v/pair-1/2
  projections and the out-projections ride as fillers inside early
  units so the PE stream is dense and ACT (exp, ~214us total - the
  steady-state bottleneck) is never starved.
- Input DMAs ordered wq,wk,xq,xk first so projections start early.
"""

import sys

import numpy as np

for _p in ("/opt/trn_rl_repo",):
    if _p not in sys.path:
        sys.path.insert(0, _p)

B, NQ, NK, C = 4, 2048, 2048, 768
H, DH = 12, 64
HPC, HB = 6, 384  # heads per core, head-block width
P = 128
KT = C // P  # 6 contraction k-tiles for projections
QCH = 512  # query-chunk width
NCH = NQ // QCH  # 4 query chunks
NKT = NK // P  # 16 key tiles
SCALE = DH**-0.5  # folded into Wq on host (exactly 0.125)
VW = DH + 1  # v block width per head incl. ones column

_prog = None
DEBUG = False


def _build():
    from contextlib import ExitStack

    import concourse.bass as bass
    import concourse.tile as tile
    from concourse import library_config, mybir
    from concourse.bacc import Bacc

    f32 = mybir.dt.float32
    bf16 = mybir.dt.bfloat16
    EXP = mybir.ActivationFunctionType.Exp

    nc = Bacc()
    xq_d = nc.declare_dram_parameter("xq", [C, NQ], bf16, isOutput=False)
    xk_d = nc.declare_dram_parameter("xk", [C, NK], bf16, isOutput=False)
    xv_d = nc.declare_dram_parameter("xv", [C, NK], bf16, isOutput=False)
    wq_d = nc.declare_dram_parameter("wq", [C, HB], bf16, isOutput=False)
    wk_d = nc.declare_dram_parameter("wk", [C, HB], bf16, isOutput=False)
    wv_d = nc.declare_dram_parameter("wv", [C, HB], bf16, isOutput=False)
    wo_d = nc.declare_dram_parameter("wo", [HB, C], bf16, isOutput=False)
    out_d = nc.declare_dram_parameter("out", [C, NQ], f32, isOutput=True)
    if DEBUG:
        dbg_d = {
            "dbg_d": nc.declare_dram_parameter("dbg_d", [1, QCH], f32, isOutput=True),
            "dbg_r": nc.declare_dram_parameter("dbg_r", [1, QCH], f32, isOutput=True),
            "dbg_rb": nc.declare_dram_parameter("dbg_rb", [DH, QCH], f32, isOutput=True),
            "dbg_qT": nc.declare_dram_parameter("dbg_qT", [P, NQ], f32, isOutput=True),
            "dbg_xn": nc.declare_dram_parameter("dbg_xn", [P, NQ], f32, isOutput=True),
        }

    with tile.TileContext(nc) as tc, ExitStack() as ctx:
        const = ctx.enter_context(tc.tile_pool(name="const", bufs=1))
        xin = ctx.enter_context(tc.tile_pool(name="xin", bufs=1))
        qk = ctx.enter_context(tc.tile_pool(name="qk", bufs=1))
        pp = ctx.enter_context(tc.tile_pool(name="pp", bufs=18))
        xnp = ctx.enter_context(tc.tile_pool(name="xnp", bufs=1))
        small = ctx.enter_context(tc.tile_pool(name="small", bufs=2))
        ost = ctx.enter_context(tc.tile_pool(name="ost", bufs=3))
        proj_ps = ctx.enter_context(tc.tile_pool(name="proj_ps", bufs=2, space="PSUM"))
        lt_ps = ctx.enter_context(tc.tile_pool(name="lt_ps", bufs=2, space="PSUM"))
        x_ps = ctx.enter_context(tc.tile_pool(name="x_ps", bufs=2, space="PSUM"))

        # gpsimd ucode library for partition_broadcast
        nc.gpsimd.load_library(library_config.attn)

        # ---- input DMAs, priority order (the sync HWDGE ring drains
        # roughly in order): pair-0 q/k weight slices, xq, xk, remaining
        # q/k weights, wv, xv, wo (needed last).
        wq_s = const.tile([P, KT, HB], bf16, tag="wq")
        wk_s = const.tile([P, KT, HB], bf16, tag="wk")
        wq_r = wq_d.rearrange("(k p) m -> p k m", p=P)
        wk_r = wk_d.rearrange("(k p) m -> p k m", p=P)
        nc.sync.dma_start(out=wq_s[:, :, 0:P], in_=wq_r[:, :, 0:P])
        nc.sync.dma_start(out=wk_s[:, :, 0:P], in_=wk_r[:, :, 0:P])

        # split each q/k tile DMA: chunk-0 columns first so the pair-0
        # chunk-0 projections (and with them the first exp) start after
        # ~1.9MB of input instead of the full 7.2MB
        xq_t, xk_t, xv_t = [], [], []
        for name, dram, lst in (("xq", xq_d, xq_t), ("xk", xk_d, xk_t)):
            for k in range(KT):
                t = xin.tile([P, NQ], bf16, tag=f"{name}_{k}", name=f"{name}_{k}")
                r = dram.rearrange("(k p) m -> p k m", p=P)
                nc.sync.dma_start(out=t[:, 0:QCH], in_=r[:, k, 0:QCH])
                lst.append(t)
        for name, dram, lst in (("xq", xq_d, xq_t), ("xk", xk_d, xk_t)):
            for k in range(KT):
                r = dram.rearrange("(k p) m -> p k m", p=P)
                nc.sync.dma_start(out=lst[k][:, QCH:NQ], in_=r[:, k, QCH:NQ])

        nc.sync.dma_start(out=wq_s[:, :, P:HB], in_=wq_r[:, :, P:HB])
        nc.sync.dma_start(out=wk_s[:, :, P:HB], in_=wk_r[:, :, P:HB])
        wv_s = const.tile([P, KT, HB], bf16, tag="wv")
        wo_s = const.tile([P, HB // P, C], bf16, tag="wo")
        nc.sync.dma_start(out=wv_s, in_=wv_d.rearrange("(k p) m -> p k m", p=P))
        for k in range(KT):
            t = xin.tile([P, NQ], bf16, tag=f"xv_{k}", name=f"xv_{k}")
            nc.sync.dma_start(
                out=t, in_=xv_d.rearrange("(k p) m -> p k m", p=P)[:, k, :]
            )
            xv_t.append(t)
        nc.sync.dma_start(out=wo_s, in_=wo_d.rearrange("(k p) m -> p k m", p=P))

        # v with a ones column per head: [128, kt, head, 65]
        v_s = const.tile([P, NKT, HPC, VW], bf16, tag="v")
        nc.vector.memset(v_s[:, :, :, DH : DH + 1], 1.0)

        qT_t = [qk.tile([P, NQ], bf16, tag=f"qT{i}", name=f"qT{i}") for i in range(3)]
        kT_t = [qk.tile([P, NQ], bf16, tag=f"kT{i}", name=f"kT{i}") for i in range(3)]
        xn_t = [xnp.tile([P, NQ], bf16, tag=f"xn{i}", name=f"xn{i}") for i in range(3)]

        # ---- small emitters used as PE fillers
        def projqk_block(w_s, src, dst, mt, j4):
            def f():
                ps = proj_ps.tile([P, QCH], f32, tag="proj", name="ps")
                for k in range(KT):
                    nc.tensor.matmul(
                        ps,
                        w_s[:, k, mt * P : (mt + 1) * P],
                        src[k][:, j4 * QCH : (j4 + 1) * QCH],
                        start=(k == 0),
                        stop=(k == KT - 1),
                    )
                nc.vector.tensor_copy(dst[:, j4 * QCH : (j4 + 1) * QCH], ps)

            return f

        def vproj_block(kt):
            def f():
                ps = proj_ps.tile([P, HB], f32, tag="proj", name="ps")
                for k in range(KT):
                    nc.tensor.matmul(
                        ps,
                        xv_t[k][:, kt * P : (kt + 1) * P],
                        wv_s[:, k, :],
                        start=(k == 0),
                        stop=(k == KT - 1),
                    )
                nc.vector.tensor_copy(
                    v_s[:, kt, :, 0:DH], ps.rearrange("p (h m) -> p h m", m=DH)
                )

            return f

        def outproj_block(j4, mt):
            def f():
                ps = proj_ps.tile([P, QCH], f32, tag="proj", name="ps")
                for k3 in range(HB // P):
                    nc.tensor.matmul(
                        ps,
                        wo_s[:, k3, mt * P : (mt + 1) * P],
                        xn_t[k3][:, j4 * QCH : (j4 + 1) * QCH],
                        start=(k3 == 0),
                        stop=(k3 == HB // P - 1),
                    )
                o = ost.tile([P, QCH], f32, tag="o", name="o")
                nc.vector.tensor_copy(o, ps)
                nc.sync.dma_start(
                    out=out_d[mt * P : (mt + 1) * P, j4 * QCH : (j4 + 1) * QCH], in_=o
                )

            return f

        # ---- attention unit = one (pair, chunk): 8 ktp blocks of
        # [QKT 4mm; 2 exps; AV block of the PREVIOUS unit; fillers].
        # The whole AV of unit u rides inside unit u+1 (uniform unit lag),
        # so AVs never gate on in-flight exps and xd PSUM slots alternate
        # between non-adjacent units.
        rows = (slice(0, DH), slice(DH, 2 * DH))
        av_carry = [[]]  # 8 pending AV-block emitters from the previous unit

        def av_block(p3, j4, ktp, p_pair, xd_pair):
            qsl = slice(j4 * QCH, (j4 + 1) * QCH)

            def f():
                for hh in range(2):
                    h = 2 * p3 + hh
                    for u in range(2):
                        kt = 2 * ktp + u
                        nc.tensor.matmul(
                            xd_pair[hh],
                            v_s[:, kt, h, :],
                            p_pair[hh][:, u * QCH : (u + 1) * QCH],
                            start=(kt == 0),
                            stop=(kt == NKT - 1),
                        )
                if ktp == NKT // 2 - 1:
                    # normalize both heads, chains interleaved so the two
                    # heads pipeline across DVE and GpSimd.
                    # custom-DVE ops mishandle nonzero input base partition:
                    # copy the denominator row to partition 0 first.
                    # h1 first: its path has the extra cross-partition DMA hop
                    dcps, rs, rbs = {}, {}, {}
                    for hh in (1, 0):
                        dcps[hh] = small.tile([1, QCH], f32, tag="dcp", name="dcp")
                        nc.vector.tensor_copy(dcps[hh], xd_pair[hh][DH : DH + 1, :])
                    for hh in (1, 0):
                        rs[hh] = small.tile([1, QCH], f32, tag="r", name="r")
                        nc.vector.reciprocal_approx_fast(rs[hh], dcps[hh])
                    for hh in (1, 0):
                        rbs[hh] = small.tile([DH, QCH], f32, tag="rb", name="rb")
                        nc.gpsimd.partition_broadcast(rbs[hh], rs[hh])
                    tmp = small.tile([DH, QCH], bf16, tag="tmp", name="tmp")
                    nc.vector.tensor_mul(tmp, xd_pair[1][0:DH, :], rbs[1])
                    nc.vector.tensor_mul(
                        xn_t[p3][0:DH, qsl], xd_pair[0][0:DH, :], rbs[0]
                    )
                    nc.sync.dma_start(out=xn_t[p3][DH : 2 * DH, qsl], in_=tmp)

            return f

        def attn_unit(p3, j4, fillers, chase=False):
            """fillers: 8 lists of emitters, one per ktp block, emitted
            after the previous unit's AV block. chase=True additionally
            runs this unit's own AVs at 1-ktp lag (for the final unit, so
            only one AV block remains after the last exp)."""
            qsl = slice(j4 * QCH, (j4 + 1) * QCH)
            xd_pair = [
                x_ps.tile([DH + 1, QCH], f32, tag="x", name=f"xd{hh}")
                for hh in range(2)
            ]
            prev_av = av_carry[0]
            my_av = []
            for ktp in range(NKT // 2):
                lts = [
                    lt_ps.tile([P, 2 * QCH], f32, tag="lt", name=f"lt{hh}")
                    for hh in range(2)
                ]
                for u in range(2):
                    kt = 2 * ktp + u
                    for hh in range(2):
                        nc.tensor.matmul(
                            lts[hh][:, u * QCH : (u + 1) * QCH],
                            kT_t[p3][rows[hh], kt * P : (kt + 1) * P],
                            qT_t[p3][rows[hh], qsl],
                            start=True,
                            stop=True,
                        )
                p_pair = []
                for hh in range(2):
                    pt = pp.tile([P, 2 * QCH], bf16, tag="p", name="pt")
                    nc.scalar.activation(pt, lts[hh], EXP)
                    p_pair.append(pt)
                # previous unit's AVs on blocks 0..6 (two on block 0) so its
                # normalize completes well before the next unit reuses xd
                if ktp == 0:
                    for g in prev_av[0:2]:
                        g()
                elif ktp <= 6 and ktp + 1 < len(prev_av):
                    prev_av[ktp + 1]()
                my_av.append(av_block(p3, j4, ktp, p_pair, xd_pair))
                if chase and ktp > 0:
                    my_av[ktp - 1]()
                for f in fillers[ktp]:
                    f()
            av_carry[0] = my_av[-1:] if chase else my_av

        def flush_av():
            for f in av_carry[0]:
                f()
            av_carry[0] = []

        def spread(emitters):
            """Distribute a list of emitters over 8 ktp blocks, in order."""
            out = [[] for _ in range(8)]
            for i, e in enumerate(emitters):
                out[min(i * 8 // max(len(emitters), 1), 7)].append(e)
            return out

        # ---- emission schedule. Region-level deps let QKT(p,c,ktp) start
        # once qT[p] chunk c and kT[p] key-block ktp//2 are projected, so
        # only qp0(j0)+kp0(j0) run before the first attention unit; every
        # other projection block rides as a filler.
        def qp(p3, j4):
            return projqk_block(wq_s, xq_t, qT_t[p3], p3, j4)

        def kp(p3, j4):
            return projqk_block(wk_s, xk_t, kT_t[p3], p3, j4)

        qp(0, 0)()
        kp(0, 0)()

        # unit (p0,c0): kp0 key-blocks just in time (QKT ktp needs
        # kp0(ktp//2)); v-projection at blocks 4-7 (xv arrives late; the
        # AVs consuming it ride in unit (p1,c0)); pair-1 j0 at the end.
        u0 = [[] for _ in range(8)]
        u0[1].append(kp(0, 1))
        u0[3].append(kp(0, 2))
        u0[5].append(kp(0, 3))
        for i in range(8):
            u0[4 + i // 2].append(vproj_block(i))
        u0[6].append(qp(1, 0))
        u0[7].append(kp(1, 0))
        attn_unit(0, 0, u0)

        # unit (p1,c0): rest of the v-projection just in time for the
        # AV(p0,c0) blocks riding here (AV ktp k at block max(k-1,0) needs
        # v tiles 2k,2k+1), rest of kp1, pair-2 j0 at the end
        u1 = [[] for _ in range(8)]
        for i in range(8, 16):
            u1[(i - 8) // 2].append(vproj_block(i))
        u1[1].append(kp(1, 1))
        u1[3].append(kp(1, 2))
        u1[5].append(kp(1, 3))
        u1[6].append(qp(2, 0))
        u1[7].append(kp(2, 0))
        attn_unit(1, 0, u1)

        # unit (p2,c0): rest of kp2, plus q projections for chunk 1
        u2 = [[] for _ in range(8)]
        u2[1].append(kp(2, 1))
        u2[3].append(kp(2, 2))
        u2[5].append(kp(2, 3))
        u2[6].append(qp(0, 1))
        u2[7].append(qp(1, 1))
        attn_unit(2, 0, u2)

        # chunks 1..3. normalize(p2,c) lands in unit (p0,c+1) block 6, so
        # out-proj(c) rides in unit (p1,c+1) blocks 1..6. q projections for
        # later chunks trickle in one unit ahead of their consumer.
        for j4 in range(1, NCH):
            ua = [[] for _ in range(8)]
            attn_unit(0, j4, ua)
            ub = [[] for _ in range(8)]
            ub[0].append(qp(2, j4))
            # in the last chunk keep out-proj mts 3..5 of chunk 2 for the
            # flush, so the PE stays warm through the final normalize
            n_op = 3 if j4 == NCH - 1 else C // P
            for mt in range(n_op):
                ub[1 + mt].append(outproj_block(j4 - 1, mt))
            attn_unit(1, j4, ub, chase=(j4 == NCH - 1))
            uc = [[] for _ in range(8)]
            if j4 < NCH - 1:
                uc[6].append(qp(0, j4 + 1))
                uc[7].append(qp(1, j4 + 1))
            attn_unit(2, j4, uc, chase=(j4 == NCH - 1))

        # flush: AV+normalize of the last unit overlapped with the held-back
        # chunk-2 out-projs (PE warm-keepers), then the final chunk's out-proj
        flush_av()
        for mt in range(3, C // P):
            outproj_block(NCH - 2, mt)()
        for mt in range(C // P):
            outproj_block(NCH - 1, mt)()

        if DEBUG:
            for nm, t in (("dbg_qT", qT_t[0]), ("dbg_xn", xn_t[0])):
                dc = ost.tile([P, NQ], f32, tag="dbgc", name="dbgc", bufs=1)
                nc.vector.tensor_copy(dc, t)
                nc.sync.dma_start(out=dbg_d[nm][:, :], in_=dc)

    nc.finalize()
    return nc


def _get_prog():
    global _prog
    if _prog is None:
        _prog = _build()
    return _prog


def _shard_inputs(query, key, value, Wq, Wk, Wv, Wo):
    from ml_dtypes import bfloat16

    in_maps = []
    for core in range(8):
        b, hg = core // 2, core % 2
        sl = slice(hg * HB, (hg + 1) * HB)
        in_maps.append(
            {
                "xq": np.ascontiguousarray(query[b].T).astype(bfloat16),
                "xk": np.ascontiguousarray(key[b].T).astype(bfloat16),
                "xv": np.ascontiguousarray(value[b].T).astype(bfloat16),
                "wq": np.ascontiguousarray((Wq[sl, :] * SCALE).T).astype(bfloat16),
                "wk": np.ascontiguousarray(Wk[sl, :].T).astype(bfloat16),
                "wv": np.ascontiguousarray(Wv[sl, :].T).astype(bfloat16),
                "wo": np.ascontiguousarray(Wo[:, sl].T).astype(bfloat16),
            }
        )
    return in_maps


def kernel(query, key, value, Wq, Wk, Wv, Wo, bo):
    query, key, value = np.asarray(query), np.asarray(key), np.asarray(value)
    Wq, Wk, Wv, Wo = np.asarray(Wq), np.asarray(Wk), np.asarray(Wv), np.asarray(Wo)
    bo = np.asarray(bo).astype(np.float32)

    from concourse.bass_utils import run_bass_kernel_spmd

    nc = _get_prog()
    in_maps = _shard_inputs(query, key, value, Wq, Wk, Wv, Wo)
    res = run_bass_kernel_spmd(nc, in_maps, list(range(8))).results

    out = np.empty((B, NQ, C), np.float32)
    for b in range(B):
        acc = res[2 * b]["out"].astype(np.float32) + res[2 * b + 1]["out"].astype(
            np.float32
        )
        out[b] = acc.T + bo[None, :]
    return out
